# revision 14
# baseline (speedup 1.0000x reference)
"""Two-layer GCN encoder on 8 Trainium2 NeuronCores (Bass/Tile).

V2 strategy (edge-parallel by destination range):
  - Sort edges by dst on the host; core k owns dst range [6400k, 6400(k+1)).
  - Degrees/normalization precomputed on host (pure edge-index metadata);
    the gather table xs_tbl = dis*x is shipped as a DRAM parameter, so the
    kernel starts gathering immediately (no degree pass, no table build).
  - Segment-sum via one-hot matmul with the one-hot as the STATIONARY
    operand: PE cost per 128-edge chunk is proportional to the feature
    width (5 for layer 1, 64 for layer 2) instead of 128.
  - Layer-1 psum [128n, 5] is transposed via PE to feed W1; layer-2 psum
    [128n, 64] is already in output orientation (no transpose needed).
  - zt is AllGathered in 4 chunks interleaved with pass-2 compute; each
    chunk is repacked into a 256B-row gather table by a strided DMA copy.
"""
import sys

sys.path.insert(0, "/opt/trn_rl_repo")

import numpy as np

from concourse import bacc, mybir, tile
from concourse import library_config
from concourse.bass_utils import run_bass_kernel_spmd

P = 128
NCORES = 8
N_NODES = 50000
RANGE = 6400                  # nodes per core (50 tiles of 128)
NT = RANGE // P               # 50 node tiles per core
NG = NCORES * NT              # 400 global node tiles
V = NCORES * RANGE            # 51200 padded table rows
HALF = 32768                  # int16 index split point
F2 = 64                       # zt cols
FX = 5                        # raw x feature count
TBLW = 128                    # table row width (fp16 -> 256B rows)
GT = 2                        # tiles per gather group
PAD_DST = 9999                # one-hot miss value for padded edge slots
NCHUNK = 4                    # zt AllGather chunks (small last chunk -> short tail)
CHUNK_TILES = (16, 16, 14, 4)

f16 = mybir.dt.float16
f32 = mybir.dt.float32
i16 = mybir.dt.int16

_prog_cache = {}


def build_program(cpt_lo, cpt_hi):
    cpt = cpt_lo + cpt_hi
    C = NT * cpt                      # dst16 columns per core
    NGRP = NT // GT
    CL = NT * cpt_lo * 8              # idx_lo columns (128/16 per chunk)
    CH = NT * cpt_hi * 8

    nc = bacc.Bacc("TRN2", target_bir_lowering=False, debug=False,
                   num_devices=NCORES)

    dst_rel = nc.declare_dram_parameter("dst_rel", [P, C], f16, isOutput=False)
    idx_lo = nc.declare_dram_parameter("idx_lo", [P, CL], i16, isOutput=False)
    idx_hi = nc.declare_dram_parameter("idx_hi", [P, CH], i16, isOutput=False)
    xs_tbl = nc.declare_dram_parameter("xs_tbl", [V, TBLW], f16, isOutput=False)
    xs_own = nc.declare_dram_parameter("xs_own", [P, NT, FX], f16, isOutput=False)
    w1 = nc.declare_dram_parameter("w1", [FX, 128], f16, isOutput=False)
    w2 = nc.declare_dram_parameter("w2", [128, F2], f16, isOutput=False)
    b1r = nc.declare_dram_parameter("b1r", [1, 128], f16, isOutput=False)
    b2bc_in = nc.declare_dram_parameter("b2bc", [P, F2], f32, isOutput=False)
    invdis_in = nc.declare_dram_parameter("invdis", [1, RANGE], f16, isOutput=False)
    dis_in = nc.declare_dram_parameter("dis_c", [P, NT], f32, isOutput=False)
    dis2_in = nc.declare_dram_parameter("dis2_c", [P, NT], f32, isOutput=False)
    iota_in = nc.declare_dram_parameter("iota_in", [P, P * cpt], f16, isOutput=False)
    ident_in = nc.declare_dram_parameter("ident_in", [P, P], f16, isOutput=False)
    out_ext = nc.declare_dram_parameter("out", [RANGE, F2], f32, isOutput=True)

    ztown_dram = nc.dram_tensor("ztown_dram", [RANGE, F2], f16)
    ztg_dram = [
        nc.dram_tensor(f"ztg{c}_dram", [NCORES * CHUNK_TILES[c] * P, F2], f16,
                       addr_space="Shared")
        for c in range(NCHUNK)
    ]
    ztglob_dram = nc.dram_tensor("ztglob_dram", [V, TBLW], f16)

    rg = [list(range(NCORES))]
    mlp = library_config.mlp

    with tile.TileContext(nc) as tc:
        with (
            tc.tile_pool(name="const", bufs=1) as const,
            tc.tile_pool(name="ohp", bufs=4) as ohp,
            tc.tile_pool(name="msgp", bufs=4) as msgp,
            tc.tile_pool(name="smallp", bufs=4) as smallp,
            tc.tile_pool(name="ps_seg", bufs=2, space="PSUM") as ps_seg,
            tc.tile_pool(name="ps_big", bufs=2, space="PSUM") as ps_big,
            tc.tile_pool(name="ps_aux", bufs=2, space="PSUM") as ps_aux,
            tc.tile_pool(name="ps_tr", bufs=2, space="PSUM") as ps_tr,
        ):
            nc.gpsimd.load_library(mlp)

            iota16 = const.tile([P, P * cpt], f16)
            nc.sync.dma_start(out=iota16[:], in_=iota_in[:])
            ident = const.tile([P, P], f16)
            nc.sync.dma_start(out=ident[:], in_=ident_in[:])
            dst16 = const.tile([P, C], f16)
            nc.sync.dma_start(out=dst16[:], in_=dst_rel[:])
            idxlo_sb = const.tile([P, CL], i16)
            nc.sync.dma_start(out=idxlo_sb[:], in_=idx_lo[:])
            idxhi_sb = const.tile([P, CH], i16)
            nc.sync.dma_start(out=idxhi_sb[:], in_=idx_hi[:])

            w1_sb = const.tile([FX, 128], f16)
            nc.sync.dma_start(out=w1_sb[:], in_=w1[:])
            b1row = const.tile([1, 128], f16)
            nc.sync.dma_start(out=b1row[:], in_=b1r[:])
            w2_sb = const.tile([128, F2], f16)
            nc.sync.dma_start(out=w2_sb[:], in_=w2[:])
            b2bc = const.tile([P, F2], f32)
            nc.sync.dma_start(out=b2bc[:], in_=b2bc_in[:])
            invdis_flat = const.tile([1, RANGE], f16)
            nc.sync.dma_start(out=invdis_flat[:], in_=invdis_in[:])
            dis_cols = const.tile([P, NT], f32)
            nc.sync.dma_start(out=dis_cols[:], in_=dis_in[:])
            dis2_cols = const.tile([P, NT], f32)
            nc.sync.dma_start(out=dis2_cols[:], in_=dis2_in[:])
            xs_own_sb = const.tile([P, NT, FX], f16)
            nc.sync.dma_start(out=xs_own_sb[:], in_=xs_own[:])

            ztf32 = const.tile([P, NT, F2], f32)

            def oh_build(oh, t):
                """Transposed one-hot for tile t: oh[p, n, c] = (dst[p,c]==n).
                Last dim of every operand is stride-1 -> DVE 2x_1p mode."""
                q, j = divmod(t, GT)
                lo0 = q * GT * cpt + j * cpt_lo
                hi0 = q * GT * cpt + GT * cpt_lo + j * cpt_hi
                ohv = oh[:].rearrange("p (n c) -> p n c", c=cpt)
                iov = iota16[:].rearrange("p (n c) -> p n c", c=cpt)
                nc.vector.tensor_tensor(
                    out=ohv[:, :, 0:cpt_lo],
                    in0=dst16[:, None, lo0:lo0 + cpt_lo].broadcast_to(
                        [P, P, cpt_lo]),
                    in1=iov[:, :, 0:cpt_lo],
                    op=mybir.AluOpType.is_equal,
                )
                nc.vector.tensor_tensor(
                    out=ohv[:, :, cpt_lo:cpt],
                    in0=dst16[:, None, hi0:hi0 + cpt_hi].broadcast_to(
                        [P, P, cpt_hi]),
                    in1=iov[:, :, cpt_lo:cpt],
                    op=mybir.AluOpType.is_equal,
                )

            def seg_matmuls(acc, oh, msg, j, width, last_stop):
                """acc[128n, 0:width] += oh_chunk.T @ msg_chunk over tile j's
                chunks; one-hot stationary, msg moving (cost ~ width)."""
                ohv = oh[:].rearrange("p (n c) -> p n c", c=cpt)
                for i in range(cpt):
                    if i < cpt_lo:
                        mcol = j * cpt_lo + i
                    else:
                        mcol = GT * cpt_lo + j * cpt_hi + (i - cpt_lo)
                    nc.tensor.matmul(
                        out=acc[:, 0:width], lhsT=ohv[:, :, i],
                        rhs=msg[:, mcol, 0:width],
                        start=(i == 0), stop=(last_stop and i == cpt - 1),
                    )

            def gathers(msg, q, table):
                nlo = GT * cpt_lo * P
                nhi = GT * cpt_hi * P
                nc.gpsimd.dma_gather(
                    msg[:, 0:GT * cpt_lo, :], table[0:HALF, :],
                    idxlo_sb[:, q * GT * cpt_lo * 8:(q + 1) * GT * cpt_lo * 8],
                    nlo, nlo, TBLW, single_packet=False,
                )
                nc.gpsimd.dma_gather(
                    msg[:, GT * cpt_lo:GT * cpt, :], table[HALF:V, :],
                    idxhi_sb[:, q * GT * cpt_hi * 8:(q + 1) * GT * cpt_hi * 8],
                    nhi, nhi, TBLW, single_packet=False,
                )

            # chunk boundaries (tile index where each chunk ends)
            chunk_end = []
            acc = 0
            for ct in CHUNK_TILES:
                acc += ct
                chunk_end.append(acc)

            def emit_cc(cix):
                r0 = (chunk_end[cix] - CHUNK_TILES[cix]) * P
                r1 = chunk_end[cix] * P
                nc.gpsimd.collective_compute(
                    "AllGather", mybir.AluOpType.bypass,
                    replica_groups=rg,
                    ins=[ztown_dram[r0:r1, :]],
                    outs=[ztg_dram[cix][:]],
                )

            def emit_repack(cix):
                # repack chunk into the 256B-row gather table
                r0 = (chunk_end[cix] - CHUNK_TILES[cix]) * P
                r1 = chunk_end[cix] * P
                nc.sync.dma_start(
                    out=ztglob_dram.ap().rearrange(
                        "(k r) f -> k r f", k=NCORES)[:, r0:r1, 0:F2],
                    in_=ztg_dram[cix].ap().rearrange(
                        "(k r) f -> k r f", k=NCORES, r=r1 - r0),
                )

            # Emit each AllGather on the Pool queue shortly after its input
            # tiles complete, and its repack on the SP queue several groups
            # later, so neither semaphore wait stalls a sequencer long.
            CC_DELAY, RP_DELAY = 2, 7
            cc_after_group, rp_after_group = {}, {}
            for cix in range(NCHUNK - 1):
                qc = (chunk_end[cix] - 1) // GT
                cc_after_group.setdefault(min(qc + CC_DELAY, NT // GT - 1),
                                          []).append(cix)
                rp_after_group.setdefault(min(qc + RP_DELAY, NT // GT - 1),
                                          []).append(cix)

            # ---------- pass 1: layer 1 -> zt table ----------
            for q in range(NGRP):
                msg = msgp.tile([P, GT * cpt, TBLW], f16, tag="msg")
                gathers(msg, q, xs_tbl)
                for j in range(GT):
                    t = q * GT + j
                    oh = ohp.tile([P, cpt * P], f16, tag="oh")
                    oh_build(oh, t)
                    g1t = ps_seg.tile([P, F2], f32, tag="seg")
                    seg_matmuls(g1t, oh, msg, j, FX, last_stop=False)
                    nc.tensor.matmul(out=g1t[:, 0:FX], lhsT=ident[:],
                                     rhs=xs_own_sb[:, t, :],
                                     start=False, stop=True)
                    s1_sb = smallp.tile([P, FX], f16, tag="s1sb")
                    nc.scalar.copy(out=s1_sb[:], in_=g1t[:, 0:FX])
                    s1tp = ps_tr.tile([FX, P], f16, tag="tr")
                    nc.tensor.transpose(out=s1tp[:], in_=s1_sb[:],
                                        identity=ident[:])
                    s1t = smallp.tile([FX, P], f16, tag="s1t")
                    nc.scalar.copy(out=s1t[:], in_=s1tp[:])
                    h1p = ps_big.tile([P, P], f32, tag="h1")
                    nc.tensor.matmul(out=h1p[:], lhsT=w1_sb[:], rhs=s1t[:],
                                     start=True, stop=False)
                    nc.tensor.matmul(out=h1p[:], lhsT=b1row[:],
                                     rhs=invdis_flat[:, t * P:(t + 1) * P],
                                     start=False, stop=True)
                    h1r = smallp.tile([P, P], f16, tag="h1r")
                    nc.scalar.activation(out=h1r[:], in_=h1p[:],
                                         func=mybir.ActivationFunctionType.Relu)
                    ztp = ps_aux.tile([P, F2], f32, tag="aux")
                    nc.tensor.matmul(out=ztp[:], lhsT=h1r[:], rhs=w2_sb[:],
                                     start=True, stop=True)
                    nc.vector.tensor_tensor(
                        out=ztf32[:, t, :], in0=ztp[:],
                        in1=dis2_cols[:, t:t + 1].to_broadcast([P, F2]),
                        op=mybir.AluOpType.mult,
                    )
                    zt16 = smallp.tile([P, F2], f16, tag="zt16")
                    nc.vector.tensor_copy(out=zt16[:], in_=ztf32[:, t, :])
                    nc.sync.dma_start(out=ztown_dram[t * P:(t + 1) * P, :],
                                      in_=zt16[:])
                for cix in cc_after_group.get(q, []):
                    emit_cc(cix)
                for cix in rp_after_group.get(q, []):
                    emit_repack(cix)
            emit_cc(NCHUNK - 1)
            emit_repack(NCHUNK - 1)

            # ---------- pass 2: layer 2 -> output ----------
            for q in range(NGRP):
                msg = msgp.tile([P, GT * cpt, TBLW], f16, tag="msg")
                gathers(msg, q, ztglob_dram)
                for j in range(GT):
                    t = q * GT + j
                    oh = ohp.tile([P, cpt * P], f16, tag="oh")
                    oh_build(oh, t)
                    g2 = ps_seg.tile([P, F2], f32, tag="seg")
                    seg_matmuls(g2, oh, msg, j, F2, last_stop=True)
                    sum_sb = smallp.tile([P, F2], f32, tag="sum")
                    nc.vector.tensor_add(out=sum_sb[:], in0=g2[:],
                                         in1=ztf32[:, t, :])
                    out_sb = smallp.tile([P, F2], f32, tag="outt")
                    nc.vector.scalar_tensor_tensor(
                        out=out_sb[:], in0=sum_sb[:],
                        scalar=dis_cols[:, t:t + 1], in1=b2bc[:],
                        op0=mybir.AluOpType.mult, op1=mybir.AluOpType.add,
                    )
                    nc.sync.dma_start(out=out_ext[t * P:(t + 1) * P, :],
                                      in_=out_sb[:])

    nc.compile()
    return nc


def _prepare_shards(src, dst):
    """Group edges by dst tile, split into lo/hi src streams, pad to uniform
    chunk counts, and emit device arrays in the group-major slot layout."""
    E = src.shape[0]
    tile_g = dst >> 7

    hi_mask0 = src >= HALF
    # order: by tile, lo stream first, stable
    sub_order = np.lexsort((np.arange(E), hi_mask0.astype(np.int8), tile_g))
    ssrc = src[sub_order]
    stile = tile_g[sub_order]
    sdst = dst[sub_order]
    hi_mask = ssrc >= HALF

    lo_counts = np.bincount(stile[~hi_mask], minlength=NG)
    hi_counts = np.bincount(stile[hi_mask], minlength=NG)
    cpt_lo = max(1, int(np.ceil(lo_counts.max() / P)))
    cpt_hi = max(1, int(np.ceil(hi_counts.max() / P)))
    cap_lo, cap_hi = cpt_lo * P, cpt_hi * P

    tile_starts = np.zeros(NG + 1, np.int64)
    np.cumsum(lo_counts + hi_counts, out=tile_starts[1:])
    pos_in_tile = np.arange(E, dtype=np.int64) - tile_starts[stile]
    within = np.where(hi_mask, pos_in_tile - lo_counts[stile], pos_in_tile)

    src_lo = np.zeros((NG, cap_lo), np.int16)          # pad -> row 0
    dst_lo = np.full((NG, cap_lo), PAD_DST, np.int32)
    src_hi = np.zeros((NG, cap_hi), np.int16)
    dst_hi = np.full((NG, cap_hi), PAD_DST, np.int32)
    lo_sel = ~hi_mask
    src_lo[stile[lo_sel], within[lo_sel]] = ssrc[lo_sel].astype(np.int16)
    dst_lo[stile[lo_sel], within[lo_sel]] = sdst[lo_sel] & 127
    src_hi[stile[hi_mask], within[hi_mask]] = (ssrc[hi_mask] - HALF).astype(
        np.int16)
    dst_hi[stile[hi_mask], within[hi_mask]] = sdst[hi_mask] & 127

    cpt = cpt_lo + cpt_hi
    NGRP = NT // GT
    dst16_dev, idxlo_dev, idxhi_dev = [], [], []

    def wrap(a):
        lin = a.reshape(-1)
        w = lin.reshape(-1, 16).T          # idx j -> [j%16, j//16]
        return np.ascontiguousarray(np.tile(w, (8, 1)))

    for k in range(NCORES):
        tl = slice(k * NT, (k + 1) * NT)
        klo_src = src_lo[tl].reshape(NGRP, GT, cpt_lo, P)
        klo_dst = dst_lo[tl].reshape(NGRP, GT, cpt_lo, P)
        khi_src = src_hi[tl].reshape(NGRP, GT, cpt_hi, P)
        khi_dst = dst_hi[tl].reshape(NGRP, GT, cpt_hi, P)

        # dst16 [P, NGRP*(GT*cpt)]; per-group cols [lo(t0) lo(t1) hi(t0) hi(t1)]
        dcols = np.concatenate(
            [klo_dst.reshape(NGRP, GT * cpt_lo, P),
             khi_dst.reshape(NGRP, GT * cpt_hi, P)], axis=1)
        d16 = dcols.transpose(2, 0, 1).reshape(P, NT * cpt).astype(np.float16)
        dst16_dev.append(np.ascontiguousarray(d16))

        idxlo_dev.append(wrap(klo_src))
        idxhi_dev.append(wrap(khi_src))

    return cpt_lo, cpt_hi, dst16_dev, idxlo_dev, idxhi_dev


def make_inputs(x, edge_index, W1, b1, W2, b2):
    x = np.asarray(x, np.float32)
    ei = np.asarray(edge_index)
    src = ei[0].astype(np.int32)
    dst = ei[1].astype(np.int32)

    cpt_lo, cpt_hi, dst16_dev, idxlo_dev, idxhi_dev = _prepare_shards(src, dst)

    # host-side degree / normalization (edge-index metadata)
    deg = np.bincount(dst, minlength=N_NODES).astype(np.float32) + 1.0
    dis = (1.0 / np.sqrt(deg)).astype(np.float32)       # all deg >= 1
    invdis = np.sqrt(deg).astype(np.float32)

    dis_pad = np.zeros(V, np.float32)
    dis_pad[:N_NODES] = dis
    xs = np.zeros((V, FX), np.float32)
    xs[:N_NODES] = x * dis[:, None]

    xs_tbl = np.zeros((V, TBLW), np.float16)
    xs_tbl[:, :FX] = xs.astype(np.float16)

    # per-core column tensors: [P, NT] with node n = t*128 + p
    dis_cols_all = dis_pad.reshape(NG, P).T             # [P, NG]
    invdis_pad = np.zeros(V, np.float32)
    invdis_pad[:N_NODES] = invdis
    xs_own_all = xs.reshape(NG, P, FX).transpose(1, 0, 2)  # [P, NG, FX]

    cpt = cpt_lo + cpt_hi
    iota = np.tile(np.repeat(np.arange(P, dtype=np.float16), cpt)[None, :],
                   (P, 1))
    ident = np.eye(P, dtype=np.float16)
    b2bc = np.tile(np.asarray(b2, np.float32)[None, :], (P, 1))
    b1r = np.asarray(b1, np.float32)[None, :].astype(np.float16)
    w1_16 = np.asarray(W1, np.float32).astype(np.float16)
    w2_16 = np.asarray(W2, np.float32).astype(np.float16)

    in_maps = []
    for k in range(NCORES):
        tl = slice(k * NT, (k + 1) * NT)
        in_maps.append({
            "dst_rel": dst16_dev[k],
            "idx_lo": idxlo_dev[k],
            "idx_hi": idxhi_dev[k],
            "xs_tbl": xs_tbl,
            "xs_own": np.ascontiguousarray(
                xs_own_all[:, tl, :]).astype(np.float16),
            "w1": w1_16, "w2": w2_16, "b1r": b1r, "b2bc": b2bc,
            "invdis": np.ascontiguousarray(
                invdis_pad[k * RANGE:(k + 1) * RANGE])[None, :].astype(
                    np.float16),
            "dis_c": np.ascontiguousarray(dis_cols_all[:, tl]),
            "dis2_c": np.ascontiguousarray(dis_cols_all[:, tl] ** 2),
            "iota_in": iota, "ident_in": ident,
        })
    return (cpt_lo, cpt_hi), in_maps


def kernel(x, edge_index, W1, b1, W2, b2):
    key, in_maps = make_inputs(x, edge_index, W1, b1, W2, b2)
    if key not in _prog_cache:
        _prog_cache[key] = build_program(*key)
    nc = _prog_cache[key]
    res = run_bass_kernel_spmd(nc, in_maps, list(range(NCORES)))
    out = np.concatenate([res.results[k]["out"] for k in range(NCORES)], axis=0)
    return out[:N_NODES]


# revision 15
# speedup vs baseline: 1.0946x; 1.0946x over previous
"""Two-layer GCN encoder on 8 Trainium2 NeuronCores (Bass/Tile).

V2 strategy (edge-parallel by destination range):
  - Sort edges by dst on the host; core k owns dst range [6400k, 6400(k+1)).
  - Degrees/normalization precomputed on host (pure edge-index metadata);
    the gather table xs_tbl = dis*x is shipped as a DRAM parameter, so the
    kernel starts gathering immediately (no degree pass, no table build).
  - Segment-sum via one-hot matmul with the one-hot as the STATIONARY
    operand: PE cost per 128-edge chunk is proportional to the feature
    width (5 for layer 1, 64 for layer 2) instead of 128.
  - Layer-1 psum [128n, 5] is transposed via PE to feed W1; layer-2 psum
    [128n, 64] is already in output orientation (no transpose needed).
  - zt is AllGathered in 4 chunks interleaved with pass-2 compute; each
    chunk is repacked into a 256B-row gather table by a strided DMA copy.
"""
import sys

sys.path.insert(0, "/opt/trn_rl_repo")

import numpy as np

from concourse import bacc, mybir, tile
from concourse import library_config
from concourse.bass_utils import run_bass_kernel_spmd

P = 128
NCORES = 8
N_NODES = 50000
RANGE = 6400                  # nodes per core (50 tiles of 128)
NT = RANGE // P               # 50 node tiles per core
NG = NCORES * NT              # 400 global node tiles
V = NCORES * RANGE            # 51200 padded table rows
HALF = 32768                  # int16 index split point
F2 = 64                       # zt cols
FX = 5                        # raw x feature count
TBLW = 128                    # table row width (fp16 -> 256B rows)
GT = 2                        # tiles per gather group
PAD_DST = 9999                # one-hot miss value for padded edge slots
NCHUNK = 4                    # zt AllGather chunks (small last chunk -> short tail)
CHUNK_TILES = (16, 16, 14, 4)

f16 = mybir.dt.float16
f32 = mybir.dt.float32
i16 = mybir.dt.int16

_prog_cache = {}


def build_program(cpt_lo, cpt_hi):
    cpt = cpt_lo + cpt_hi
    C = NT * cpt                      # dst16 columns per core
    NGRP = NT // GT
    CL = NT * cpt_lo * 8              # idx_lo columns (128/16 per chunk)
    CH = NT * cpt_hi * 8

    nc = bacc.Bacc("TRN2", target_bir_lowering=False, debug=False,
                   num_devices=NCORES)

    dst_rel = nc.declare_dram_parameter("dst_rel", [P, C], f16, isOutput=False)
    idx_lo = nc.declare_dram_parameter("idx_lo", [P, CL], i16, isOutput=False)
    idx_hi = nc.declare_dram_parameter("idx_hi", [P, CH], i16, isOutput=False)
    xs_tbl = nc.declare_dram_parameter("xs_tbl", [V, TBLW], f16, isOutput=False)
    xs_own = nc.declare_dram_parameter("xs_own", [P, NT, FX], f16, isOutput=False)
    w1 = nc.declare_dram_parameter("w1", [FX, 128], f16, isOutput=False)
    w2 = nc.declare_dram_parameter("w2", [128, F2], f16, isOutput=False)
    b1r = nc.declare_dram_parameter("b1r", [1, 128], f16, isOutput=False)
    b2bc_in = nc.declare_dram_parameter("b2bc", [P, F2], f32, isOutput=False)
    invdis_in = nc.declare_dram_parameter("invdis", [1, RANGE], f16, isOutput=False)
    dis_in = nc.declare_dram_parameter("dis_c", [P, NT], f32, isOutput=False)
    dis2_in = nc.declare_dram_parameter("dis2_c", [P, NT], f32, isOutput=False)
    iota_in = nc.declare_dram_parameter("iota_in", [P, P * cpt], f16, isOutput=False)
    ident_in = nc.declare_dram_parameter("ident_in", [P, P], f16, isOutput=False)
    out_ext = nc.declare_dram_parameter("out", [RANGE, F2], f32, isOutput=True)

    ztown_dram = nc.dram_tensor("ztown_dram", [RANGE, F2], f16)
    ztg_dram = [
        nc.dram_tensor(f"ztg{c}_dram", [NCORES * CHUNK_TILES[c] * P, F2], f16,
                       addr_space="Shared")
        for c in range(NCHUNK)
    ]
    ztglob_dram = nc.dram_tensor("ztglob_dram", [V, TBLW], f16)

    rg = [list(range(NCORES))]
    mlp = library_config.mlp

    with tile.TileContext(nc) as tc:
        with (
            tc.tile_pool(name="const", bufs=1) as const,
            tc.tile_pool(name="ohp", bufs=4) as ohp,
            tc.tile_pool(name="msgp", bufs=4) as msgp,
            tc.tile_pool(name="smallp", bufs=4) as smallp,
            tc.tile_pool(name="ps_seg", bufs=2, space="PSUM") as ps_seg,
            tc.tile_pool(name="ps_big", bufs=2, space="PSUM") as ps_big,
            tc.tile_pool(name="ps_aux", bufs=2, space="PSUM") as ps_aux,
            tc.tile_pool(name="ps_tr", bufs=2, space="PSUM") as ps_tr,
        ):
            nc.gpsimd.load_library(mlp)

            iota16 = const.tile([P, P * cpt], f16)
            nc.sync.dma_start(out=iota16[:], in_=iota_in[:])
            ident = const.tile([P, P], f16)
            nc.sync.dma_start(out=ident[:], in_=ident_in[:])
            dst16 = const.tile([P, C], f16)
            nc.sync.dma_start(out=dst16[:], in_=dst_rel[:])
            idxlo_sb = const.tile([P, CL], i16)
            nc.sync.dma_start(out=idxlo_sb[:], in_=idx_lo[:])
            idxhi_sb = const.tile([P, CH], i16)
            nc.sync.dma_start(out=idxhi_sb[:], in_=idx_hi[:])

            w1_sb = const.tile([FX, 128], f16)
            nc.sync.dma_start(out=w1_sb[:], in_=w1[:])
            b1row = const.tile([1, 128], f16)
            nc.sync.dma_start(out=b1row[:], in_=b1r[:])
            w2_sb = const.tile([128, F2], f16)
            nc.sync.dma_start(out=w2_sb[:], in_=w2[:])
            b2bc = const.tile([P, F2], f32)
            nc.sync.dma_start(out=b2bc[:], in_=b2bc_in[:])
            invdis_flat = const.tile([1, RANGE], f16)
            nc.sync.dma_start(out=invdis_flat[:], in_=invdis_in[:])
            dis_cols = const.tile([P, NT], f32)
            nc.sync.dma_start(out=dis_cols[:], in_=dis_in[:])
            dis2_cols = const.tile([P, NT], f32)
            nc.sync.dma_start(out=dis2_cols[:], in_=dis2_in[:])
            xs_own_sb = const.tile([P, NT, FX], f16)
            nc.sync.dma_start(out=xs_own_sb[:], in_=xs_own[:])

            ztf32 = const.tile([P, NT, F2], f32)

            def oh_build(oh, t):
                """Transposed one-hot for tile t: oh[p, n, c] = (dst[p,c]==n).
                Last dim of every operand is stride-1 -> DVE 2x_1p mode."""
                q, j = divmod(t, GT)
                lo0 = q * GT * cpt + j * cpt_lo
                hi0 = q * GT * cpt + GT * cpt_lo + j * cpt_hi
                ohv = oh[:].rearrange("p (n c) -> p n c", c=cpt)
                iov = iota16[:].rearrange("p (n c) -> p n c", c=cpt)
                nc.vector.tensor_tensor(
                    out=ohv[:, :, 0:cpt_lo],
                    in0=dst16[:, None, lo0:lo0 + cpt_lo].broadcast_to(
                        [P, P, cpt_lo]),
                    in1=iov[:, :, 0:cpt_lo],
                    op=mybir.AluOpType.is_equal,
                )
                nc.vector.tensor_tensor(
                    out=ohv[:, :, cpt_lo:cpt],
                    in0=dst16[:, None, hi0:hi0 + cpt_hi].broadcast_to(
                        [P, P, cpt_hi]),
                    in1=iov[:, :, cpt_lo:cpt],
                    op=mybir.AluOpType.is_equal,
                )

            def seg_matmuls(acc, oh, msg, j, width, last_stop):
                """acc[128n, 0:width] += oh_chunk.T @ msg_chunk over tile j's
                chunks; one-hot stationary, msg moving (cost ~ width)."""
                ohv = oh[:].rearrange("p (n c) -> p n c", c=cpt)
                for i in range(cpt):
                    if i < cpt_lo:
                        mcol = j * cpt_lo + i
                    else:
                        mcol = GT * cpt_lo + j * cpt_hi + (i - cpt_lo)
                    nc.tensor.matmul(
                        out=acc[:, 0:width], lhsT=ohv[:, :, i],
                        rhs=msg[:, mcol, 0:width],
                        start=(i == 0), stop=(last_stop and i == cpt - 1),
                    )

            def gathers(msg, q, table):
                nlo = GT * cpt_lo * P
                nhi = GT * cpt_hi * P
                nc.gpsimd.dma_gather(
                    msg[:, 0:GT * cpt_lo, :], table[0:HALF, :],
                    idxlo_sb[:, q * GT * cpt_lo * 8:(q + 1) * GT * cpt_lo * 8],
                    nlo, nlo, TBLW, single_packet=False,
                )
                nc.gpsimd.dma_gather(
                    msg[:, GT * cpt_lo:GT * cpt, :], table[HALF:V, :],
                    idxhi_sb[:, q * GT * cpt_hi * 8:(q + 1) * GT * cpt_hi * 8],
                    nhi, nhi, TBLW, single_packet=False,
                )

            # chunk boundaries (tile index where each chunk ends)
            chunk_end = []
            acc = 0
            for ct in CHUNK_TILES:
                acc += ct
                chunk_end.append(acc)

            def emit_cc(cix):
                r0 = (chunk_end[cix] - CHUNK_TILES[cix]) * P
                r1 = chunk_end[cix] * P
                nc.gpsimd.collective_compute(
                    "AllGather", mybir.AluOpType.bypass,
                    replica_groups=rg,
                    ins=[ztown_dram[r0:r1, :]],
                    outs=[ztg_dram[cix][:]],
                )

            def emit_repack(cix):
                # repack chunk into the 256B-row gather table
                r0 = (chunk_end[cix] - CHUNK_TILES[cix]) * P
                r1 = chunk_end[cix] * P
                nc.sync.dma_start(
                    out=ztglob_dram.ap().rearrange(
                        "(k r) f -> k r f", k=NCORES)[:, r0:r1, 0:F2],
                    in_=ztg_dram[cix].ap().rearrange(
                        "(k r) f -> k r f", k=NCORES, r=r1 - r0),
                )

            # Emit each AllGather on the Pool queue shortly after its input
            # tiles complete, and its repack on the SP queue several groups
            # later, so neither semaphore wait stalls a sequencer long.
            CC_DELAY, RP_DELAY = 6, 8
            cc_after_group, rp_after_group = {}, {}
            for cix in range(NCHUNK - 1):
                qc = (chunk_end[cix] - 1) // GT
                cc_after_group.setdefault(min(qc + CC_DELAY, NT // GT - 1),
                                          []).append(cix)
                rp_after_group.setdefault(min(qc + RP_DELAY, NT // GT - 1),
                                          []).append(cix)

            # ---------- pass 1: layer 1 -> zt table ----------
            for q in range(NGRP):
                msg = msgp.tile([P, GT * cpt, TBLW], f16, tag="msg")
                gathers(msg, q, xs_tbl)
                for j in range(GT):
                    t = q * GT + j
                    oh = ohp.tile([P, cpt * P], f16, tag="oh")
                    oh_build(oh, t)
                    g1t = ps_seg.tile([P, F2], f32, tag="seg")
                    seg_matmuls(g1t, oh, msg, j, FX, last_stop=False)
                    nc.tensor.matmul(out=g1t[:, 0:FX], lhsT=ident[:],
                                     rhs=xs_own_sb[:, t, :],
                                     start=False, stop=True)
                    s1_sb = smallp.tile([P, FX], f16, tag="s1sb")
                    nc.scalar.copy(out=s1_sb[:], in_=g1t[:, 0:FX])
                    s1tp = ps_tr.tile([FX, P], f16, tag="tr")
                    nc.tensor.transpose(out=s1tp[:], in_=s1_sb[:],
                                        identity=ident[:])
                    s1t = smallp.tile([FX, P], f16, tag="s1t")
                    nc.scalar.copy(out=s1t[:], in_=s1tp[:])
                    h1p = ps_big.tile([P, P], f32, tag="h1")
                    nc.tensor.matmul(out=h1p[:], lhsT=w1_sb[:], rhs=s1t[:],
                                     start=True, stop=False)
                    nc.tensor.matmul(out=h1p[:], lhsT=b1row[:],
                                     rhs=invdis_flat[:, t * P:(t + 1) * P],
                                     start=False, stop=True)
                    h1r = smallp.tile([P, P], f16, tag="h1r")
                    nc.scalar.activation(out=h1r[:], in_=h1p[:],
                                         func=mybir.ActivationFunctionType.Relu)
                    ztp = ps_aux.tile([P, F2], f32, tag="aux")
                    nc.tensor.matmul(out=ztp[:], lhsT=h1r[:], rhs=w2_sb[:],
                                     start=True, stop=True)
                    nc.vector.tensor_tensor(
                        out=ztf32[:, t, :], in0=ztp[:],
                        in1=dis2_cols[:, t:t + 1].to_broadcast([P, F2]),
                        op=mybir.AluOpType.mult,
                    )
                    zt16 = smallp.tile([P, F2], f16, tag="zt16")
                    nc.vector.tensor_copy(out=zt16[:], in_=ztf32[:, t, :])
                    nc.sync.dma_start(out=ztown_dram[t * P:(t + 1) * P, :],
                                      in_=zt16[:])
                for cix in cc_after_group.get(q, []):
                    emit_cc(cix)
                for cix in rp_after_group.get(q, []):
                    emit_repack(cix)
            emit_cc(NCHUNK - 1)
            emit_repack(NCHUNK - 1)

            # ---------- pass 2: layer 2 -> output ----------
            for q in range(NGRP):
                msg = msgp.tile([P, GT * cpt, TBLW], f16, tag="msg")
                gathers(msg, q, ztglob_dram)
                for j in range(GT):
                    t = q * GT + j
                    oh = ohp.tile([P, cpt * P], f16, tag="oh")
                    oh_build(oh, t)
                    g2 = ps_seg.tile([P, F2], f32, tag="seg")
                    seg_matmuls(g2, oh, msg, j, F2, last_stop=True)
                    sum_sb = smallp.tile([P, F2], f32, tag="sum")
                    nc.vector.tensor_add(out=sum_sb[:], in0=g2[:],
                                         in1=ztf32[:, t, :])
                    out_sb = smallp.tile([P, F2], f32, tag="outt")
                    nc.vector.scalar_tensor_tensor(
                        out=out_sb[:], in0=sum_sb[:],
                        scalar=dis_cols[:, t:t + 1], in1=b2bc[:],
                        op0=mybir.AluOpType.mult, op1=mybir.AluOpType.add,
                    )
                    nc.sync.dma_start(out=out_ext[t * P:(t + 1) * P, :],
                                      in_=out_sb[:])

    nc.compile()
    return nc


def _prepare_shards(src, dst):
    """Group edges by dst tile, split into lo/hi src streams, pad to uniform
    chunk counts, and emit device arrays in the group-major slot layout."""
    E = src.shape[0]
    tile_g = dst >> 7

    hi_mask0 = src >= HALF
    # order: by tile, lo stream first, stable
    sub_order = np.lexsort((np.arange(E), hi_mask0.astype(np.int8), tile_g))
    ssrc = src[sub_order]
    stile = tile_g[sub_order]
    sdst = dst[sub_order]
    hi_mask = ssrc >= HALF

    lo_counts = np.bincount(stile[~hi_mask], minlength=NG)
    hi_counts = np.bincount(stile[hi_mask], minlength=NG)
    cpt_lo = max(1, int(np.ceil(lo_counts.max() / P)))
    cpt_hi = max(1, int(np.ceil(hi_counts.max() / P)))
    cap_lo, cap_hi = cpt_lo * P, cpt_hi * P

    tile_starts = np.zeros(NG + 1, np.int64)
    np.cumsum(lo_counts + hi_counts, out=tile_starts[1:])
    pos_in_tile = np.arange(E, dtype=np.int64) - tile_starts[stile]
    within = np.where(hi_mask, pos_in_tile - lo_counts[stile], pos_in_tile)

    src_lo = np.zeros((NG, cap_lo), np.int16)          # pad -> row 0
    dst_lo = np.full((NG, cap_lo), PAD_DST, np.int32)
    src_hi = np.zeros((NG, cap_hi), np.int16)
    dst_hi = np.full((NG, cap_hi), PAD_DST, np.int32)
    lo_sel = ~hi_mask
    src_lo[stile[lo_sel], within[lo_sel]] = ssrc[lo_sel].astype(np.int16)
    dst_lo[stile[lo_sel], within[lo_sel]] = sdst[lo_sel] & 127
    src_hi[stile[hi_mask], within[hi_mask]] = (ssrc[hi_mask] - HALF).astype(
        np.int16)
    dst_hi[stile[hi_mask], within[hi_mask]] = sdst[hi_mask] & 127

    cpt = cpt_lo + cpt_hi
    NGRP = NT // GT
    dst16_dev, idxlo_dev, idxhi_dev = [], [], []

    def wrap(a):
        lin = a.reshape(-1)
        w = lin.reshape(-1, 16).T          # idx j -> [j%16, j//16]
        return np.ascontiguousarray(np.tile(w, (8, 1)))

    for k in range(NCORES):
        tl = slice(k * NT, (k + 1) * NT)
        klo_src = src_lo[tl].reshape(NGRP, GT, cpt_lo, P)
        klo_dst = dst_lo[tl].reshape(NGRP, GT, cpt_lo, P)
        khi_src = src_hi[tl].reshape(NGRP, GT, cpt_hi, P)
        khi_dst = dst_hi[tl].reshape(NGRP, GT, cpt_hi, P)

        # dst16 [P, NGRP*(GT*cpt)]; per-group cols [lo(t0) lo(t1) hi(t0) hi(t1)]
        dcols = np.concatenate(
            [klo_dst.reshape(NGRP, GT * cpt_lo, P),
             khi_dst.reshape(NGRP, GT * cpt_hi, P)], axis=1)
        d16 = dcols.transpose(2, 0, 1).reshape(P, NT * cpt).astype(np.float16)
        dst16_dev.append(np.ascontiguousarray(d16))

        idxlo_dev.append(wrap(klo_src))
        idxhi_dev.append(wrap(khi_src))

    return cpt_lo, cpt_hi, dst16_dev, idxlo_dev, idxhi_dev


def make_inputs(x, edge_index, W1, b1, W2, b2):
    x = np.asarray(x, np.float32)
    ei = np.asarray(edge_index)
    src = ei[0].astype(np.int32)
    dst = ei[1].astype(np.int32)

    cpt_lo, cpt_hi, dst16_dev, idxlo_dev, idxhi_dev = _prepare_shards(src, dst)

    # host-side degree / normalization (edge-index metadata)
    deg = np.bincount(dst, minlength=N_NODES).astype(np.float32) + 1.0
    dis = (1.0 / np.sqrt(deg)).astype(np.float32)       # all deg >= 1
    invdis = np.sqrt(deg).astype(np.float32)

    dis_pad = np.zeros(V, np.float32)
    dis_pad[:N_NODES] = dis
    xs = np.zeros((V, FX), np.float32)
    xs[:N_NODES] = x * dis[:, None]

    xs_tbl = np.zeros((V, TBLW), np.float16)
    xs_tbl[:, :FX] = xs.astype(np.float16)

    # per-core column tensors: [P, NT] with node n = t*128 + p
    dis_cols_all = dis_pad.reshape(NG, P).T             # [P, NG]
    invdis_pad = np.zeros(V, np.float32)
    invdis_pad[:N_NODES] = invdis
    xs_own_all = xs.reshape(NG, P, FX).transpose(1, 0, 2)  # [P, NG, FX]

    cpt = cpt_lo + cpt_hi
    iota = np.tile(np.repeat(np.arange(P, dtype=np.float16), cpt)[None, :],
                   (P, 1))
    ident = np.eye(P, dtype=np.float16)
    b2bc = np.tile(np.asarray(b2, np.float32)[None, :], (P, 1))
    b1r = np.asarray(b1, np.float32)[None, :].astype(np.float16)
    w1_16 = np.asarray(W1, np.float32).astype(np.float16)
    w2_16 = np.asarray(W2, np.float32).astype(np.float16)

    in_maps = []
    for k in range(NCORES):
        tl = slice(k * NT, (k + 1) * NT)
        in_maps.append({
            "dst_rel": dst16_dev[k],
            "idx_lo": idxlo_dev[k],
            "idx_hi": idxhi_dev[k],
            "xs_tbl": xs_tbl,
            "xs_own": np.ascontiguousarray(
                xs_own_all[:, tl, :]).astype(np.float16),
            "w1": w1_16, "w2": w2_16, "b1r": b1r, "b2bc": b2bc,
            "invdis": np.ascontiguousarray(
                invdis_pad[k * RANGE:(k + 1) * RANGE])[None, :].astype(
                    np.float16),
            "dis_c": np.ascontiguousarray(dis_cols_all[:, tl]),
            "dis2_c": np.ascontiguousarray(dis_cols_all[:, tl] ** 2),
            "iota_in": iota, "ident_in": ident,
        })
    return (cpt_lo, cpt_hi), in_maps


def kernel(x, edge_index, W1, b1, W2, b2):
    key, in_maps = make_inputs(x, edge_index, W1, b1, W2, b2)
    if key not in _prog_cache:
        _prog_cache[key] = build_program(*key)
    nc = _prog_cache[key]
    res = run_bass_kernel_spmd(nc, in_maps, list(range(NCORES)))
    out = np.concatenate([res.results[k]["out"] for k in range(NCORES)], axis=0)
    return out[:N_NODES]
